# revision 33
# baseline (speedup 1.0000x reference)
"""Trainium2 Bass kernel for nn_Attention_2851858284976.

Dense transformer attention block, b=8 n=1024 dim=1024 heads=16.
Sharding: pure data parallel — one batch element per NeuronCore (8 cores).

Per-core math (batch element x of shape (n, dim)):
  Y = x @ w_qkv^T                              (n, 3*dim)
  Z = Y.reshape(49152, 64)   # raw reshape: rows are (token, col-block) pairs
  Q = Z[0:16384], K = Z[16384:32768], V = Z[32768:49152], each (16, 1024, 64)
  per head: P^T = exp(scale * K_h @ Q_h^T)     (softmax along the partition axis)
            [O^T; Zs*64] = [V_h | 1*64]^T @ P^T  (ones cols replicate the denom)
            oT_h = O^T * (1/Zs)
  out = (oT stacked).T @ w_out^T + b_out

Q/K SBUF layout: Z row r = 48*i + c0 decomposes as c0 = 16*a + b (a<3,
b<16), r = 16*T + b with T = 3*i + a.  Heads are 1024 = 64*16 tokens, so
head h's tokens t = 16*u + b map to a contiguous T-run [64h, 64h+64) x all
b.  QKb[d, T, b] (b innermost) gives contiguous head windows with IDENTITY
token order (col = 16*dT + b) and phase-1A scatters that write [i, b-run]
blocks instead of 96B-strided single elements.  After phase 1A, one
SBUF->SBUF DMA duplicates QK onto partitions 64-127 so the K=64-contraction
score matmuls can run packed two-per-slot with row tiling (tile_position
auto-derived from the base partition; emission order A0,A1,B0,B1 keeps
zero ACT gaps with a 2-deep sps pool).

Schedule (phases sequential — interleaving phase-1 leftovers into the
consume loop measured WORSE on HW due to PE-FIFO stalls + HAM throttle):
  phase 1A (2 dense sweeps) -> dup DMA -> B it5/it6/it7 (V region, with
  pre-produced exps filling an 8-deep pt queue during it6/it7) -> consume
  stream (PV + spill-divide, produce-ahead pairs) -> out projection.
"""
import numpy as np
import ml_dtypes

import concourse.bass as bass
import concourse.mybir as mybir
from concourse import bacc
from concourse.tile import TileContext
from concourse.bass_utils import run_bass_kernel_spmd

N_CORES = 8
N = 1024          # tokens
DIM = 1024
E3 = 3 * DIM      # qkv projection width
H = 16            # heads
HD = 64           # head dim
SCALE = HD ** -0.5
TSPAN = 2049      # T = 3*i + a, i < 683 -> T in [0, 2049)
VROWS = 48 * 342  # 16416: V Z-rows (16384) plus 32 slack
PTBUFS = 10       # pt queue depth (produce-ahead)

F32 = mybir.dt.float32
BF = mybir.dt.bfloat16
FT = mybir.ActivationFunctionType


def build():
    nc = bacc.Bacc("TRN2", target_bir_lowering=False, num_devices=N_CORES)
    xt = nc.declare_dram_parameter("xt", [DIM, N], BF, isOutput=False)
    wqkvt = nc.declare_dram_parameter("wqkvt", [DIM, E3], BF, isOutput=False)
    woutt = nc.declare_dram_parameter("woutt", [DIM, DIM], BF, isOutput=False)
    bias = nc.declare_dram_parameter("bias", [1, DIM], F32, isOutput=False)
    outp = nc.declare_dram_parameter("out", [N, DIM], F32, isOutput=True)

    with TileContext(nc) as tc:
        with tc.tile_pool(name="dram", bufs=1, space="DRAM") as dpool, \
             tc.tile_pool(name="singles", bufs=1) as singles:
            # packed V buffer: flat row q = Z row 32768+q = v of head q//1024
            vbuf = dpool.tile([VROWS, HD], BF)
            vb3 = vbuf.rearrange("(a c) d -> a c d", c=48)   # (342, 48, 64)

            oT = singles.tile([128, 8, N], BF)    # [64*(h%2)+dd, h//2, i]
            biasrep = singles.tile([128, DIM], F32)
            vh0 = singles.tile([128, 8, 2 * HD], BF)
            vh1 = singles.tile([128, 8, 2 * HD], BF)
            vh2 = singles.tile([128, 8, 2 * HD], BF)
            vh3 = singles.tile([128, 8, 2 * HD], BF)
            vhs = [vh0, vh1, vh2, vh3]
            WOT = singles.tile([128, 8, DIM], BF)

            with tc.tile_pool(name="qk", bufs=1) as qkpool:
                # QKb[d, T, b] = Z[16*T + b, d]; partitions 64-127 = DMA dup
                QKb = qkpool.tile([128, TSPAN, 16], BF)
                # scatter view: T = 3*i + a, b = 2*b2 + dl
                QKs = QKb.rearrange("p (i a) (b2 dl) -> p i a b2 dl",
                                    a=3, dl=2)

                def qt_sl(h, ic, hf):
                    w = QKb[64 * hf:64 * hf + 64,
                            64 * h + 32 * ic:64 * h + 32 * ic + 32, :]
                    return w.rearrange("p T b -> p (T b)")

                def kt_sl(h, jt, hf):
                    w = QKb[64 * hf:64 * hf + 64,
                            1024 + 64 * h + 8 * jt:1024 + 64 * h + 8 * jt + 8, :]
                    return w.rearrange("p T b -> p (T b)")

                with tc.tile_pool(name="p1", bufs=1) as p1:
                    XT = p1.tile([128, 8, N], BF)
                    # WT[:, kt, a, e'] = wqkvt rows kt*128.., col a*1024+e'
                    WT = p1.tile([128, 8, 3, N], BF)
                    # sweep-1 needs only XT cols 0:384; load those first so
                    # the first matmul group isn't gated on the full 8MB.
                    for tk in range(2):
                        cl, ch = (0, 384) if tk == 0 else (384, N)
                        for kt in range(8):
                            nc.sync.dma_start(
                                out=XT[:, kt, cl:ch],
                                in_=xt[kt * 128:(kt + 1) * 128, cl:ch])
                    # WT in (a, col-half) consumption order, 0.5MB pieces.
                    for a in range(3):
                        for ch in range(2):
                            for kh in range(2):
                                nc.sync.dma_start(
                                    out=WT[:, 4 * kh:4 * kh + 4, a,
                                           ch * 512:(ch + 1) * 512],
                                    in_=wqkvt[kh * 512:(kh + 1) * 512,
                                              a * N + ch * 512:
                                              a * N + (ch + 1) * 512
                                              ].rearrange(
                                                  "(kt p) e -> p kt e", p=128))
                    nc.sync.dma_start(out=biasrep,
                                      in_=bias[:].to_broadcast((128, DIM)))
                    nc.sync.dma_start(
                        out=WOT, in_=woutt[:].rearrange("(a p) e -> p a e", p=128))
                    # [V | ones*64]: ones half replicates the softmax denom.
                    for v in vhs:
                        nc.vector.memset(v, 1.0)

                    SWEEPS = [(0, 384), (384, 683)]

                    def a_group_mms(ps, nblk, a, m0, i_lo, i_hi):
                        cnt = i_hi - i_lo
                        for mm in range(nblk):
                            m = m0 + mm
                            for kt in range(8):
                                nc.tensor.matmul(
                                    ps[:, mm, 0:cnt],
                                    lhsT=WT[:, kt, a, m * 128:(m + 1) * 128],
                                    rhs=XT[:, kt, i_lo:i_hi],
                                    start=(kt == 0), stop=(kt == 7))

                    def a_group_scatter(ps, nblk, a, m0, i_lo, i_hi):
                        cnt = i_hi - i_lo
                        for dl in range(2):  # c0 parity
                            src = ps[64 * dl:64 * dl + 64, :, 0:cnt]
                            src = src.rearrange("p m i -> p i m")
                            dst = QKs[0:64, i_lo:i_hi, a, m0:m0 + nblk, dl]
                            if dl == 0:
                                nc.scalar.copy(dst, src)
                            else:
                                nc.vector.tensor_copy(dst, src)

                    # ---------- phase 1A: two dense sweeps ----------------
                    # per-sweep dup DMA copies QK to partitions 64-127 (for
                    # packed pairs); sweep-1's dup overlaps sweep 2 on the
                    # DMA engines so the first produces aren't gated on it.
                    with tc.tile_pool(name="psA0", bufs=2, space="PSUM") as psA0:
                        for i_lo, i_hi in SWEEPS:
                            for a in range(3):
                                for mh in range(2):
                                    ps4 = psA0.tile([128, 4, 512], F32)
                                    a_group_mms(ps4, 4, a, 4 * mh, i_lo, i_hi)
                                    a_group_scatter(ps4, 4, a, 4 * mh,
                                                    i_lo, i_hi)
                            nc.sync.dma_start(
                                out=QKb[64:128, 3 * i_lo:3 * i_hi, :],
                                in_=QKb[0:64, 3 * i_lo:3 * i_hi, :])

                    # ---------- phase 1B tiles ----------------------------
                    def emit_b_tile(bit, ec, pBpool, psBpool):
                        ps = psBpool.tile([128, 512], F32)
                        for kt in range(8):
                            nc.tensor.matmul(
                                ps,
                                lhsT=XT[:, kt, bit * 128:(bit + 1) * 128],
                                rhs=WT[:, kt, (ec * 512) // N,
                                       (ec * 512) % N:(ec * 512) % N + 512],
                                start=(kt == 0), stop=(kt == 7))
                        st = pBpool.tile([128, 8, HD], BF)
                        nc.vector.tensor_copy(
                            st, ps.rearrange("p (b d) -> p b d", d=HD))
                        # V rows: q = 48*T + c0 - 32768
                        if ec <= 3:
                            plo = 43 if bit == 5 else 0
                            nc.sync.dma_start(
                                out=vb3[bit * 128 + plo - 683:
                                        (bit + 1) * 128 - 683,
                                        ec * 8 + 16: ec * 8 + 24, :],
                                in_=st[plo:128, :, :])
                        else:
                            plo = 42 if bit == 5 else 0
                            nc.sync.dma_start(
                                out=vb3[bit * 128 + plo - 682:
                                        (bit + 1) * 128 - 682,
                                        (ec - 4) * 8: (ec - 4) * 8 + 8, :],
                                in_=st[plo:128, :, :])

                    def load_v(h):
                        nc.sync.dma_start(
                            out=vhs[h % 4][:, :, 0:HD],
                            in_=vbuf[h * N:(h + 1) * N, :].rearrange(
                                "(t p) d -> p t d", p=128))

                    with tc.tile_pool(name="pB", bufs=2) as pB, \
                         tc.tile_pool(name="pt", bufs=PTBUFS) as ptpool, \
                         tc.tile_pool(name="sp", bufs=1) as sppool, \
                         tc.tile_pool(name="rz", bufs=1) as rzpool, \
                         tc.tile_pool(name="sps", bufs=2,
                                      space="PSUM") as spsum:
                        steps = [(h, jt) for h in range(H) for jt in range(8)]
                        state = {"next_p": 0, "ops": None, "limit": 128}
                        pts = []

                        def produce_pair():
                            # packed pair: jt even on PE rows 0-63 (QK lo),
                            # jt odd on rows 64-127 (QK dup).  Emission
                            # order A0,A1,B0,B1 lets B0 run concurrent with
                            # A1 (disjoint row groups) while tile B reuses
                            # the slot freed by the pair-ago exp.
                            h, jt0 = steps[state["next_p"]]
                            sA = spsum.tile([128, 2, 512], F32, tag="sps")
                            if jt0 % 4 == 0:
                                # HAM warm-keeper (dep-free WOT operand);
                                # overwritten by the scores start=True.
                                nc.tensor.matmul(
                                    sA[0:128, 0, 0:128],
                                    lhsT=WOT[:, 0, 0:128],
                                    rhs=WOT[:, 0, 0:128],
                                    start=True, stop=True)
                            for ic in range(2):
                                nc.tensor.matmul(
                                    sA[:, ic, :], lhsT=kt_sl(h, jt0, 0),
                                    rhs=qt_sl(h, ic, 0),
                                    start=True, stop=True)
                            sB = spsum.tile([128, 2, 512], F32, tag="sps")
                            for ic in range(2):
                                nc.tensor.matmul(
                                    sB[:, ic, :], lhsT=kt_sl(h, jt0 + 1, 1),
                                    rhs=qt_sl(h, ic, 1),
                                    start=True, stop=True)
                            for sx in (sA, sB):
                                pt = ptpool.tile([128, 2, 512], BF, tag="pt")
                                nc.scalar.activation(pt, sx, FT.Exp,
                                                     scale=SCALE)
                                pts.append(pt)
                            state["next_p"] += 2

                        def fill_pts():
                            while (state["next_p"] < min(state["limit"],
                                                         len(steps))
                                   and len(pts) <= PTBUFS - 2):
                                produce_pair()

                        # ---- B it5 (V heads 0-3) + pre-produce ----------
                        with tc.tile_pool(name="psB5", bufs=4,
                                          space="PSUM") as psB5:
                            for ec in range(6):
                                emit_b_tile(5, ec, pB, psB5)
                                fill_pts()
                            for hv in range(4):
                                load_v(hv)


                        # ---- consume stream; B it6/it7 spliced into the
                        # first 12 steps (B tiles gate nothing the exps
                        # need, so no PE-FIFO circularity; V loads for h>=4
                        # trigger naturally after their B tiles).
                        with tc.tile_pool(name="ops", bufs=1,
                                          space="PSUM") as opsum:

                            def consume_step(s):
                                h, jt = steps[s]
                                po, hf = 64 * (h % 2), h // 2
                                if jt == 0:
                                    ops0 = opsum.tile([128, 512], F32,
                                                      tag="ops0")
                                    ops1 = opsum.tile([128, 512], F32,
                                                      tag="ops1")
                                    state["ops"] = (ops0, ops1)
                                ops = state["ops"]
                                assert pts, f"pt queue empty at step {s}"
                                pt_cur = pts.pop(0)
                                fill_pts()
                                for ic in range(2):
                                    nc.tensor.matmul(
                                        ops[ic],
                                        lhsT=vhs[h % 4][:, jt, :],
                                        rhs=pt_cur[:, ic, :],
                                        start=(jt == 0), stop=(jt == 7),
                                        skip_group_check=True)
                                if jt == 7 and h + 4 < H:
                                    load_v(h + 4)
                                if jt == 7:
                                    # spill [O; Z] to SBUF to free the ops
                                    # tags fast; recip+mul off the critical
                                    # path (custom recip needs UNSHIFTED
                                    # partitions — regular copies shift).
                                    spz = sppool.tile([64, 2, 512], F32,
                                                      tag="spz")
                                    spo = sppool.tile([64, 2, 512], BF,
                                                      tag="spo")
                                    for ic in range(2):
                                        nc.vector.tensor_copy(
                                            spz[:, ic, :], ops[ic][64:128, :])
                                        nc.vector.tensor_copy(
                                            spo[:, ic, :], ops[ic][0:64, :])
                                    rzs = rzpool.tile([64, 2, 512], F32,
                                                      tag="rzs")
                                    nc.vector.reciprocal_approx_fast(rzs, spz)
                                    nc.vector.tensor_mul(
                                        oT[po:po + 64, hf, :].rearrange(
                                            "p (i c) -> p i c", c=512),
                                        spo, rzs)

                            s = 0
                            with tc.tile_pool(name="psB2", bufs=2,
                                              space="PSUM") as psB2:
                                for bit in (6, 7):
                                    for ec in range(6):
                                        consume_step(s); s += 1
                                        emit_b_tile(bit, ec, pB, psB2)
                            while s < len(steps):
                                consume_step(s); s += 1

                # ---------- phase 3: out = oT.T @ w_out^T + b ----------
                with tc.tile_pool(name="p3st", bufs=4) as p3st, \
                     tc.tile_pool(name="ps3", bufs=4, space="PSUM") as ps3p:
                    # HAM warm-keepers: cover the PE idle while the last
                    # head's divide chain runs on DVE, so the out-projection
                    # starts at full clock.
                    wps = ps3p.tile([128, 512], F32, tag="warm")
                    for _ in range(3):
                        nc.tensor.matmul(wps, lhsT=WOT[:, 0, 0:128],
                                         rhs=WOT[:, 0, 0:512],
                                         start=True, stop=True)
                    for it in range(8):
                        for ec in range(2):
                            rps = ps3p.tile([128, 512], F32)
                            for ct in range(8):
                                nc.tensor.matmul(
                                    rps,
                                    lhsT=oT[:, ct, it * 128:(it + 1) * 128],
                                    rhs=WOT[:, ct, ec * 512:(ec + 1) * 512],
                                    start=(ct == 0), stop=(ct == 7))
                            ost = p3st.tile([128, 512], F32)
                            nc.vector.tensor_add(
                                ost, rps, biasrep[:, ec * 512:(ec + 1) * 512])
                            nc.sync.dma_start(
                                out=outp[it * 128:(it + 1) * 128,
                                         ec * 512:(ec + 1) * 512],
                                in_=ost)

    nc.finalize()
    return nc


_CACHE = {}


def _get_nc():
    if "nc" not in _CACHE:
        _CACHE["nc"] = build()
    return _CACHE["nc"]


def make_in_maps(x, w_qkv, w_out, b_out):
    bf = ml_dtypes.bfloat16
    wqkvt = np.ascontiguousarray(np.asarray(w_qkv, dtype=np.float32).T).astype(bf)
    woutt = np.ascontiguousarray(np.asarray(w_out, dtype=np.float32).T).astype(bf)
    bias = np.ascontiguousarray(np.asarray(b_out, dtype=np.float32).reshape(1, DIM))
    x = np.asarray(x, dtype=np.float32)
    return [
        {
            "xt": np.ascontiguousarray(x[b].T).astype(bf),
            "wqkvt": wqkvt,
            "woutt": woutt,
            "bias": bias,
        }
        for b in range(N_CORES)
    ]


def kernel(x, w_qkv, w_out, b_out):
    nc = _get_nc()
    in_maps = make_in_maps(x, w_qkv, w_out, b_out)
    res = run_bass_kernel_spmd(nc, in_maps, core_ids=list(range(N_CORES)))
    return np.stack(
        [res.results[b]["out"] for b in range(N_CORES)], axis=0
    ).astype(np.float32)


# revision 34
# speedup vs baseline: 1.1799x; 1.1799x over previous
"""Trainium2 Bass kernel for nn_Attention_2851858284976.

Dense transformer attention block, b=8 n=1024 dim=1024 heads=16.
Sharding: pure data parallel — one batch element per NeuronCore (8 cores).

Per-core math (batch element x of shape (n, dim)):
  Y = x @ w_qkv^T                              (n, 3*dim)
  Z = Y.reshape(49152, 64)   # raw reshape: rows are (token, col-block) pairs
  Q = Z[0:16384], K = Z[16384:32768], V = Z[32768:49152], each (16, 1024, 64)
  per head: P^T = exp(scale * K_h @ Q_h^T)     (softmax along the partition axis)
            [O^T; Zs*64] = [V_h | 1*64]^T @ P^T  (ones cols replicate the denom)
            oT_h = O^T * (1/Zs)
  out = (oT stacked).T @ w_out^T + b_out

Q/K SBUF layout: Z row r = 48*i + c0 decomposes as c0 = 16*a + b (a<3,
b<16), r = 16*T + b with T = 3*i + a.  Heads are 1024 = 64*16 tokens, so
head h's tokens t = 16*u + b map to a contiguous T-run [64h, 64h+64) x all
b.  QKb[d, T, b] (b innermost) gives contiguous head windows with IDENTITY
token order (col = 16*dT + b) and phase-1A scatters that write [i, b-run]
blocks instead of 96B-strided single elements.  After phase 1A, one
SBUF->SBUF DMA duplicates QK onto partitions 64-127 so the K=64-contraction
score matmuls can run packed two-per-slot with row tiling (tile_position
auto-derived from the base partition; emission order A0,A1,B0,B1 keeps
zero ACT gaps with a 2-deep sps pool).

Schedule (phases sequential — interleaving phase-1 leftovers into the
consume loop measured WORSE on HW due to PE-FIFO stalls + HAM throttle):
  phase 1A (2 dense sweeps) -> dup DMA -> B it5/it6/it7 (V region, with
  pre-produced exps filling an 8-deep pt queue during it6/it7) -> consume
  stream (PV + spill-divide, produce-ahead pairs) -> out projection.
"""
import numpy as np
import ml_dtypes

import concourse.bass as bass
import concourse.mybir as mybir
from concourse import bacc
from concourse.tile import TileContext
from concourse.bass_utils import run_bass_kernel_spmd

N_CORES = 8
N = 1024          # tokens
DIM = 1024
E3 = 3 * DIM      # qkv projection width
H = 16            # heads
HD = 64           # head dim
SCALE = HD ** -0.5
TSPAN = 2049      # T = 3*i + a, i < 683 -> T in [0, 2049)
VROWS = 48 * 342  # 16416: V Z-rows (16384) plus 32 slack
PTBUFS = 10       # pt queue depth (produce-ahead)

F32 = mybir.dt.float32
BF = mybir.dt.bfloat16
FT = mybir.ActivationFunctionType


def build():
    nc = bacc.Bacc("TRN2", target_bir_lowering=False, num_devices=N_CORES)
    xt = nc.declare_dram_parameter("xt", [DIM, N], BF, isOutput=False)
    wqkvt = nc.declare_dram_parameter("wqkvt", [DIM, E3], BF, isOutput=False)
    woutt = nc.declare_dram_parameter("woutt", [DIM, DIM], BF, isOutput=False)
    bias = nc.declare_dram_parameter("bias", [1, DIM], F32, isOutput=False)
    outp = nc.declare_dram_parameter("out", [N, DIM], F32, isOutput=True)

    with TileContext(nc) as tc:
        with tc.tile_pool(name="dram", bufs=1, space="DRAM") as dpool, \
             tc.tile_pool(name="singles", bufs=1) as singles:
            # packed V buffer: flat row q = Z row 32768+q = v of head q//1024
            vbuf = dpool.tile([VROWS, HD], BF)
            vb3 = vbuf.rearrange("(a c) d -> a c d", c=48)   # (342, 48, 64)

            oT = singles.tile([128, 8, N], BF)    # [64*(h%2)+dd, h//2, i]
            biasrep = singles.tile([128, DIM], F32)
            vh0 = singles.tile([128, 8, 2 * HD], BF)
            vh1 = singles.tile([128, 8, 2 * HD], BF)
            vh2 = singles.tile([128, 8, 2 * HD], BF)
            vh3 = singles.tile([128, 8, 2 * HD], BF)
            vhs = [vh0, vh1, vh2, vh3]
            WOT = singles.tile([128, 8, DIM], BF)

            with tc.tile_pool(name="qk", bufs=1) as qkpool:
                # QKb[d, T, b] = Z[16*T + b, d]; partitions 64-127 = DMA dup
                QKb = qkpool.tile([128, TSPAN, 16], BF)
                # scatter view: T = 3*i + a, b = 2*b2 + dl
                QKs = QKb.rearrange("p (i a) (b2 dl) -> p i a b2 dl",
                                    a=3, dl=2)

                def qt_sl(h, ic, hf):
                    w = QKb[64 * hf:64 * hf + 64,
                            64 * h + 32 * ic:64 * h + 32 * ic + 32, :]
                    return w.rearrange("p T b -> p (T b)")

                def kt_sl(h, jt, hf):
                    w = QKb[64 * hf:64 * hf + 64,
                            1024 + 64 * h + 8 * jt:1024 + 64 * h + 8 * jt + 8, :]
                    return w.rearrange("p T b -> p (T b)")

                with tc.tile_pool(name="p1", bufs=1) as p1:
                    XT = p1.tile([128, 8, N], BF)
                    # WT[:, kt, a, e'] = wqkvt rows kt*128.., col a*1024+e'
                    WT = p1.tile([128, 8, 3, N], BF)
                    # sweep-1 needs only XT cols 0:384; load those first so
                    # the first matmul group isn't gated on the full 8MB.
                    for tk in range(2):
                        cl, ch = (0, 384) if tk == 0 else (384, N)
                        for kt in range(8):
                            nc.sync.dma_start(
                                out=XT[:, kt, cl:ch],
                                in_=xt[kt * 128:(kt + 1) * 128, cl:ch])
                    # WT in (a, col-half) consumption order, 0.5MB pieces.
                    for a in range(3):
                        for ch in range(2):
                            for kh in range(2):
                                nc.sync.dma_start(
                                    out=WT[:, 4 * kh:4 * kh + 4, a,
                                           ch * 512:(ch + 1) * 512],
                                    in_=wqkvt[kh * 512:(kh + 1) * 512,
                                              a * N + ch * 512:
                                              a * N + (ch + 1) * 512
                                              ].rearrange(
                                                  "(kt p) e -> p kt e", p=128))
                    nc.sync.dma_start(out=biasrep,
                                      in_=bias[:].to_broadcast((128, DIM)))
                    nc.sync.dma_start(
                        out=WOT, in_=woutt[:].rearrange("(a p) e -> p a e", p=128))
                    # [V | ones*64]: ones half replicates the softmax denom.
                    for v in vhs:
                        nc.vector.memset(v, 1.0)

                    SWEEPS = [(0, 384), (384, 683)]

                    def a_group_mms(ps, nblk, a, m0, i_lo, i_hi):
                        cnt = i_hi - i_lo
                        for mm in range(nblk):
                            m = m0 + mm
                            for kt in range(8):
                                nc.tensor.matmul(
                                    ps[:, mm, 0:cnt],
                                    lhsT=WT[:, kt, a, m * 128:(m + 1) * 128],
                                    rhs=XT[:, kt, i_lo:i_hi],
                                    start=(kt == 0), stop=(kt == 7))

                    def a_group_scatter(ps, nblk, a, m0, i_lo, i_hi):
                        cnt = i_hi - i_lo
                        for dl in range(2):  # c0 parity
                            src = ps[64 * dl:64 * dl + 64, :, 0:cnt]
                            src = src.rearrange("p m i -> p i m")
                            dst = QKs[0:64, i_lo:i_hi, a, m0:m0 + nblk, dl]
                            if dl == 0:
                                nc.scalar.copy(dst, src)
                            else:
                                nc.vector.tensor_copy(dst, src)

                    # ---------- phase 1A: two dense sweeps ----------------
                    # per-sweep dup DMA copies QK to partitions 64-127 (for
                    # packed pairs); sweep-1's dup overlaps sweep 2 on the
                    # DMA engines so the first produces aren't gated on it.
                    with tc.tile_pool(name="psA0", bufs=2, space="PSUM") as psA0:
                        for i_lo, i_hi in SWEEPS:
                            for a in range(3):
                                for mh in range(2):
                                    ps4 = psA0.tile([128, 4, 512], F32)
                                    a_group_mms(ps4, 4, a, 4 * mh, i_lo, i_hi)
                                    a_group_scatter(ps4, 4, a, 4 * mh,
                                                    i_lo, i_hi)
                            nc.sync.dma_start(
                                out=QKb[64:128, 3 * i_lo:3 * i_hi, :],
                                in_=QKb[0:64, 3 * i_lo:3 * i_hi, :])

                    # ---------- phase 1B tiles ----------------------------
                    def emit_b_tile(bit, ec, pBpool, psBpool):
                        ps = psBpool.tile([128, 512], F32)
                        for kt in range(8):
                            nc.tensor.matmul(
                                ps,
                                lhsT=XT[:, kt, bit * 128:(bit + 1) * 128],
                                rhs=WT[:, kt, (ec * 512) // N,
                                       (ec * 512) % N:(ec * 512) % N + 512],
                                start=(kt == 0), stop=(kt == 7))
                        st = pBpool.tile([128, 8, HD], BF)
                        nc.vector.tensor_copy(
                            st, ps.rearrange("p (b d) -> p b d", d=HD))
                        # V rows: q = 48*T + c0 - 32768
                        if ec <= 3:
                            plo = 43 if bit == 5 else 0
                            nc.sync.dma_start(
                                out=vb3[bit * 128 + plo - 683:
                                        (bit + 1) * 128 - 683,
                                        ec * 8 + 16: ec * 8 + 24, :],
                                in_=st[plo:128, :, :])
                        else:
                            plo = 42 if bit == 5 else 0
                            nc.sync.dma_start(
                                out=vb3[bit * 128 + plo - 682:
                                        (bit + 1) * 128 - 682,
                                        (ec - 4) * 8: (ec - 4) * 8 + 8, :],
                                in_=st[plo:128, :, :])

                    def load_v(h):
                        nc.sync.dma_start(
                            out=vhs[h % 4][:, :, 0:HD],
                            in_=vbuf[h * N:(h + 1) * N, :].rearrange(
                                "(t p) d -> p t d", p=128))

                    with tc.tile_pool(name="pB", bufs=2) as pB, \
                         tc.tile_pool(name="pt", bufs=PTBUFS) as ptpool, \
                         tc.tile_pool(name="sp", bufs=1) as sppool, \
                         tc.tile_pool(name="rz", bufs=1) as rzpool, \
                         tc.tile_pool(name="sps", bufs=2,
                                      space="PSUM") as spsum:
                        steps = [(h, jt) for h in range(H) for jt in range(8)]
                        state = {"next_p": 0, "ops": None, "limit": 128}
                        pts = []

                        def produce_pair():
                            # packed pair: jt even on PE rows 0-63 (QK lo),
                            # jt odd on rows 64-127 (QK dup).  Emission
                            # order A0,A1,B0,B1 lets B0 run concurrent with
                            # A1 (disjoint row groups) while tile B reuses
                            # the slot freed by the pair-ago exp.
                            h, jt0 = steps[state["next_p"]]
                            sA = spsum.tile([128, 2, 512], F32, tag="sps")
                            for ic in range(2):
                                nc.tensor.matmul(
                                    sA[:, ic, :], lhsT=kt_sl(h, jt0, 0),
                                    rhs=qt_sl(h, ic, 0),
                                    start=True, stop=True)
                            sB = spsum.tile([128, 2, 512], F32, tag="sps")
                            for ic in range(2):
                                nc.tensor.matmul(
                                    sB[:, ic, :], lhsT=kt_sl(h, jt0 + 1, 1),
                                    rhs=qt_sl(h, ic, 1),
                                    start=True, stop=True)
                            for sx in (sA, sB):
                                pt = ptpool.tile([128, 2, 512], BF, tag="pt")
                                nc.scalar.activation(pt, sx, FT.Exp,
                                                     scale=SCALE)
                                pts.append(pt)
                            state["next_p"] += 2

                        def fill_pts():
                            while (state["next_p"] < min(state["limit"],
                                                         len(steps))
                                   and len(pts) <= PTBUFS - 2):
                                produce_pair()

                        # ---- B it5 (V heads 0-3) + pre-produce ----------
                        with tc.tile_pool(name="psB5", bufs=4,
                                          space="PSUM") as psB5:
                            for ec in range(6):
                                emit_b_tile(5, ec, pB, psB5)
                                fill_pts()
                            for hv in range(4):
                                load_v(hv)


                        # ---- consume stream; B it6/it7 spliced into the
                        # first 12 steps (B tiles gate nothing the exps
                        # need, so no PE-FIFO circularity; V loads for h>=4
                        # trigger naturally after their B tiles).
                        with tc.tile_pool(name="ops", bufs=1,
                                          space="PSUM") as opsum:

                            def consume_step(s):
                                h, jt = steps[s]
                                po, hf = 64 * (h % 2), h // 2
                                if jt == 0:
                                    ops0 = opsum.tile([128, 512], F32,
                                                      tag="ops0")
                                    ops1 = opsum.tile([128, 512], F32,
                                                      tag="ops1")
                                    state["ops"] = (ops0, ops1)
                                ops = state["ops"]
                                assert pts, f"pt queue empty at step {s}"
                                pt_cur = pts.pop(0)
                                fill_pts()
                                for ic in range(2):
                                    nc.tensor.matmul(
                                        ops[ic],
                                        lhsT=vhs[h % 4][:, jt, :],
                                        rhs=pt_cur[:, ic, :],
                                        start=(jt == 0), stop=(jt == 7),
                                        skip_group_check=True)
                                if jt == 7 and h + 4 < H:
                                    load_v(h + 4)
                                if jt == 7:
                                    # spill [O; Z] to SBUF to free the ops
                                    # tags fast; recip+mul off the critical
                                    # path (custom recip needs UNSHIFTED
                                    # partitions — regular copies shift).
                                    spz = sppool.tile([64, 2, 512], F32,
                                                      tag="spz")
                                    spo = sppool.tile([64, 2, 512], BF,
                                                      tag="spo")
                                    for ic in range(2):
                                        nc.vector.tensor_copy(
                                            spz[:, ic, :], ops[ic][64:128, :])
                                        nc.vector.tensor_copy(
                                            spo[:, ic, :], ops[ic][0:64, :])
                                    rzs = rzpool.tile([64, 2, 512], F32,
                                                      tag="rzs")
                                    nc.vector.reciprocal_approx_fast(rzs, spz)
                                    nc.vector.tensor_mul(
                                        oT[po:po + 64, hf, :].rearrange(
                                            "p (i c) -> p i c", c=512),
                                        spo, rzs)

                            s = 0
                            with tc.tile_pool(name="psB2", bufs=2,
                                              space="PSUM") as psB2:
                                for bit in (6, 7):
                                    for ec in range(6):
                                        consume_step(s); s += 1
                                        emit_b_tile(bit, ec, pB, psB2)
                            while s < len(steps):
                                consume_step(s); s += 1

                # ---------- phase 3: out = oT.T @ w_out^T + b ----------
                with tc.tile_pool(name="p3st", bufs=4) as p3st, \
                     tc.tile_pool(name="ps3", bufs=4, space="PSUM") as ps3p:
                    # HAM warm-keepers: cover the PE idle while the last
                    # head's divide chain runs on DVE, so the out-projection
                    # starts at full clock.
                    wps = ps3p.tile([128, 512], F32, tag="warm")
                    for _ in range(3):
                        nc.tensor.matmul(wps, lhsT=WOT[:, 0, 0:128],
                                         rhs=WOT[:, 0, 0:512],
                                         start=True, stop=True)
                    for it in range(8):
                        for ec in range(2):
                            rps = ps3p.tile([128, 512], F32)
                            for ct in range(8):
                                nc.tensor.matmul(
                                    rps,
                                    lhsT=oT[:, ct, it * 128:(it + 1) * 128],
                                    rhs=WOT[:, ct, ec * 512:(ec + 1) * 512],
                                    start=(ct == 0), stop=(ct == 7))
                            ost = p3st.tile([128, 512], F32)
                            nc.vector.tensor_add(
                                ost, rps, biasrep[:, ec * 512:(ec + 1) * 512])
                            nc.sync.dma_start(
                                out=outp[it * 128:(it + 1) * 128,
                                         ec * 512:(ec + 1) * 512],
                                in_=ost)

    nc.finalize()
    return nc


_CACHE = {}


def _get_nc():
    if "nc" not in _CACHE:
        _CACHE["nc"] = build()
    return _CACHE["nc"]


def make_in_maps(x, w_qkv, w_out, b_out):
    bf = ml_dtypes.bfloat16
    wqkvt = np.ascontiguousarray(np.asarray(w_qkv, dtype=np.float32).T).astype(bf)
    woutt = np.ascontiguousarray(np.asarray(w_out, dtype=np.float32).T).astype(bf)
    bias = np.ascontiguousarray(np.asarray(b_out, dtype=np.float32).reshape(1, DIM))
    x = np.asarray(x, dtype=np.float32)
    return [
        {
            "xt": np.ascontiguousarray(x[b].T).astype(bf),
            "wqkvt": wqkvt,
            "woutt": woutt,
            "bias": bias,
        }
        for b in range(N_CORES)
    ]


def kernel(x, w_qkv, w_out, b_out):
    nc = _get_nc()
    in_maps = make_in_maps(x, w_qkv, w_out, b_out)
    res = run_bass_kernel_spmd(nc, in_maps, core_ids=list(range(N_CORES)))
    return np.stack(
        [res.results[b]["out"] for b in range(N_CORES)], axis=0
    ).astype(np.float32)


# revision 37
# speedup vs baseline: 1.2047x; 1.0211x over previous
"""Trainium2 Bass kernel for nn_Attention_2851858284976.

Dense transformer attention block, b=8 n=1024 dim=1024 heads=16.
Sharding: pure data parallel — one batch element per NeuronCore (8 cores).

Per-core math (batch element x of shape (n, dim)):
  Y = x @ w_qkv^T                              (n, 3*dim)
  Z = Y.reshape(49152, 64)   # raw reshape: rows are (token, col-block) pairs
  Q = Z[0:16384], K = Z[16384:32768], V = Z[32768:49152], each (16, 1024, 64)
  per head: P^T = exp(scale * K_h @ Q_h^T)     (softmax along the partition axis)
            [O^T; Zs*64] = [V_h | 1*64]^T @ P^T  (ones cols replicate the denom)
            oT_h = O^T * (1/Zs)
  out = (oT stacked).T @ w_out^T + b_out

Q/K SBUF layout: Z row r = 48*i + c0 decomposes as c0 = 16*a + b (a<3,
b<16), r = 16*T + b with T = 3*i + a.  Heads are 1024 = 64*16 tokens, so
head h's tokens t = 16*u + b map to a contiguous T-run [64h, 64h+64) x all
b.  QKb[d, T, b] (b innermost) gives contiguous head windows with IDENTITY
token order (col = 16*dT + b) and phase-1A scatters that write [i, b-run]
blocks instead of 96B-strided single elements.  After phase 1A, one
SBUF->SBUF DMA duplicates QK onto partitions 64-127 so the K=64-contraction
score matmuls can run packed two-per-slot with row tiling (tile_position
auto-derived from the base partition; emission order A0,A1,B0,B1 keeps
zero ACT gaps with a 2-deep sps pool).

Schedule (phases sequential — interleaving phase-1 leftovers into the
consume loop measured WORSE on HW due to PE-FIFO stalls + HAM throttle):
  phase 1A (2 dense sweeps) -> dup DMA -> B it5/it6/it7 (V region, with
  pre-produced exps filling an 8-deep pt queue during it6/it7) -> consume
  stream (PV + spill-divide, produce-ahead pairs) -> out projection.
"""
import numpy as np
import ml_dtypes

import concourse.bass as bass
import concourse.mybir as mybir
from concourse import bacc
from concourse.tile import TileContext
from concourse.bass_utils import run_bass_kernel_spmd

N_CORES = 8
N = 1024          # tokens
DIM = 1024
E3 = 3 * DIM      # qkv projection width
H = 16            # heads
HD = 64           # head dim
SCALE = HD ** -0.5
TSPAN = 2049      # T = 3*i + a, i < 683 -> T in [0, 2049)
VROWS = 48 * 342  # 16416: V Z-rows (16384) plus 32 slack
PTBUFS = 11       # pt queue depth (produce-ahead)

F32 = mybir.dt.float32
BF = mybir.dt.bfloat16
FT = mybir.ActivationFunctionType


def build():
    nc = bacc.Bacc("TRN2", target_bir_lowering=False, num_devices=N_CORES)
    xt = nc.declare_dram_parameter("xt", [DIM, N], BF, isOutput=False)
    wqkvt = nc.declare_dram_parameter("wqkvt", [DIM, E3], BF, isOutput=False)
    woutt = nc.declare_dram_parameter("woutt", [DIM, DIM], BF, isOutput=False)
    bias = nc.declare_dram_parameter("bias", [1, DIM], BF, isOutput=False)
    outp = nc.declare_dram_parameter("out", [N, DIM], F32, isOutput=True)

    with TileContext(nc) as tc:
        with tc.tile_pool(name="dram", bufs=1, space="DRAM") as dpool, \
             tc.tile_pool(name="singles", bufs=1) as singles:
            # packed V buffer: flat row q = Z row 32768+q = v of head q//1024
            vbuf = dpool.tile([VROWS, HD], BF)
            vb3 = vbuf.rearrange("(a c) d -> a c d", c=48)   # (342, 48, 64)

            oT = singles.tile([128, 8, N], BF)    # [64*(h%2)+dd, h//2, i]
            biasrep = singles.tile([128, DIM], BF)
            vh0 = singles.tile([128, 8, 2 * HD], BF)
            vh1 = singles.tile([128, 8, 2 * HD], BF)
            vh2 = singles.tile([128, 8, 2 * HD], BF)
            vh3 = singles.tile([128, 8, 2 * HD], BF)
            vhs = [vh0, vh1, vh2, vh3]
            WOT = singles.tile([128, 8, DIM], BF)

            with tc.tile_pool(name="qk", bufs=1) as qkpool:
                # QKb[d, T, b] = Z[16*T + b, d]; partitions 64-127 = DMA dup
                QKb = qkpool.tile([128, TSPAN, 16], BF)
                # scatter view: T = 3*i + a, b = 2*b2 + dl
                QKs = QKb.rearrange("p (i a) (b2 dl) -> p i a b2 dl",
                                    a=3, dl=2)

                def qt_sl(h, ic, hf):
                    w = QKb[64 * hf:64 * hf + 64,
                            64 * h + 32 * ic:64 * h + 32 * ic + 32, :]
                    return w.rearrange("p T b -> p (T b)")

                def kt_sl(h, jt, hf):
                    w = QKb[64 * hf:64 * hf + 64,
                            1024 + 64 * h + 8 * jt:1024 + 64 * h + 8 * jt + 8, :]
                    return w.rearrange("p T b -> p (T b)")

                with tc.tile_pool(name="p1", bufs=1) as p1:
                    XT = p1.tile([128, 8, N], BF)
                    # WT[:, kt, a, e'] = wqkvt rows kt*128.., col a*1024+e'
                    WT = p1.tile([128, 8, 3, N], BF)
                    # sweep-1 needs only XT cols 0:384; load those first so
                    # the first matmul group isn't gated on the full 8MB.
                    for tk in range(2):
                        cl, ch = (0, 384) if tk == 0 else (384, N)
                        for kt in range(8):
                            nc.sync.dma_start(
                                out=XT[:, kt, cl:ch],
                                in_=xt[kt * 128:(kt + 1) * 128, cl:ch])
                    # WT in (a, col-half) consumption order, 0.5MB pieces.
                    for a in range(3):
                        for ch in range(2):
                            for kh in range(2):
                                nc.sync.dma_start(
                                    out=WT[:, 4 * kh:4 * kh + 4, a,
                                           ch * 512:(ch + 1) * 512],
                                    in_=wqkvt[kh * 512:(kh + 1) * 512,
                                              a * N + ch * 512:
                                              a * N + (ch + 1) * 512
                                              ].rearrange(
                                                  "(kt p) e -> p kt e", p=128))
                    nc.sync.dma_start(out=biasrep,
                                      in_=bias[:].to_broadcast((128, DIM)))
                    nc.sync.dma_start(
                        out=WOT, in_=woutt[:].rearrange("(a p) e -> p a e", p=128))
                    # [V | ones*64]: ones half replicates the softmax denom.
                    for v in vhs:
                        nc.vector.memset(v, 1.0)

                    SWEEPS = [(0, 384), (384, 683)]

                    def a_group_mms(ps, nblk, a, m0, i_lo, i_hi):
                        cnt = i_hi - i_lo
                        for mm in range(nblk):
                            m = m0 + mm
                            for kt in range(8):
                                nc.tensor.matmul(
                                    ps[:, mm, 0:cnt],
                                    lhsT=WT[:, kt, a, m * 128:(m + 1) * 128],
                                    rhs=XT[:, kt, i_lo:i_hi],
                                    start=(kt == 0), stop=(kt == 7))

                    def a_group_scatter(ps, nblk, a, m0, i_lo, i_hi):
                        cnt = i_hi - i_lo
                        for dl in range(2):  # c0 parity
                            src = ps[64 * dl:64 * dl + 64, :, 0:cnt]
                            src = src.rearrange("p m i -> p i m")
                            dst = QKs[0:64, i_lo:i_hi, a, m0:m0 + nblk, dl]
                            if dl == 0:
                                nc.scalar.copy(dst, src)
                            else:
                                nc.vector.tensor_copy(dst, src)

                    # ---------- phase 1A: two dense sweeps ----------------
                    # per-sweep dup DMA copies QK to partitions 64-127 (for
                    # packed pairs); sweep-1's dup overlaps sweep 2 on the
                    # DMA engines so the first produces aren't gated on it.
                    with tc.tile_pool(name="psA0", bufs=2, space="PSUM") as psA0:
                        for i_lo, i_hi in SWEEPS:
                            for a in range(3):
                                for mh in range(2):
                                    ps4 = psA0.tile([128, 4, 512], F32)
                                    a_group_mms(ps4, 4, a, 4 * mh, i_lo, i_hi)
                                    a_group_scatter(ps4, 4, a, 4 * mh,
                                                    i_lo, i_hi)
                            nc.sync.dma_start(
                                out=QKb[64:128, 3 * i_lo:3 * i_hi, :],
                                in_=QKb[0:64, 3 * i_lo:3 * i_hi, :])

                    # ---------- phase 1B tiles ----------------------------
                    def emit_b_tile(bit, ec, pBpool, psBpool):
                        ps = psBpool.tile([128, 512], F32)
                        for kt in range(8):
                            nc.tensor.matmul(
                                ps,
                                lhsT=XT[:, kt, bit * 128:(bit + 1) * 128],
                                rhs=WT[:, kt, (ec * 512) // N,
                                       (ec * 512) % N:(ec * 512) % N + 512],
                                start=(kt == 0), stop=(kt == 7))
                        st = pBpool.tile([128, 8, HD], BF)
                        nc.vector.tensor_copy(
                            st, ps.rearrange("p (b d) -> p b d", d=HD))
                        # V rows: q = 48*T + c0 - 32768
                        if ec <= 3:
                            plo = 43 if bit == 5 else 0
                            nc.sync.dma_start(
                                out=vb3[bit * 128 + plo - 683:
                                        (bit + 1) * 128 - 683,
                                        ec * 8 + 16: ec * 8 + 24, :],
                                in_=st[plo:128, :, :])
                        else:
                            plo = 42 if bit == 5 else 0
                            nc.sync.dma_start(
                                out=vb3[bit * 128 + plo - 682:
                                        (bit + 1) * 128 - 682,
                                        (ec - 4) * 8: (ec - 4) * 8 + 8, :],
                                in_=st[plo:128, :, :])

                    def load_v(h):
                        nc.sync.dma_start(
                            out=vhs[h % 4][:, :, 0:HD],
                            in_=vbuf[h * N:(h + 1) * N, :].rearrange(
                                "(t p) d -> p t d", p=128))

                    with tc.tile_pool(name="pB", bufs=2) as pB, \
                         tc.tile_pool(name="pt", bufs=PTBUFS) as ptpool, \
                         tc.tile_pool(name="sp", bufs=1) as sppool, \
                         tc.tile_pool(name="rz", bufs=1) as rzpool, \
                         tc.tile_pool(name="sps", bufs=2,
                                      space="PSUM") as spsum:
                        steps = [(h, jt) for h in range(H) for jt in range(8)]
                        state = {"next_p": 0, "ops": None, "limit": 128}
                        pts = []

                        def produce_pair():
                            # packed pair: jt even on PE rows 0-63 (QK lo),
                            # jt odd on rows 64-127 (QK dup).  Emission
                            # order A0,A1,B0,B1 lets B0 run concurrent with
                            # A1 (disjoint row groups) while tile B reuses
                            # the slot freed by the pair-ago exp.
                            h, jt0 = steps[state["next_p"]]
                            sA = spsum.tile([128, 2, 512], F32, tag="sps")
                            for ic in range(2):
                                nc.tensor.matmul(
                                    sA[:, ic, :], lhsT=kt_sl(h, jt0, 0),
                                    rhs=qt_sl(h, ic, 0),
                                    start=True, stop=True)
                            sB = spsum.tile([128, 2, 512], F32, tag="sps")
                            for ic in range(2):
                                nc.tensor.matmul(
                                    sB[:, ic, :], lhsT=kt_sl(h, jt0 + 1, 1),
                                    rhs=qt_sl(h, ic, 1),
                                    start=True, stop=True)
                            for sx in (sA, sB):
                                pt = ptpool.tile([128, 2, 512], BF, tag="pt")
                                nc.scalar.activation(pt, sx, FT.Exp,
                                                     scale=SCALE)
                                pts.append(pt)
                            state["next_p"] += 2

                        def fill_pts():
                            while (state["next_p"] < min(state["limit"],
                                                         len(steps))
                                   and len(pts) <= PTBUFS - 2):
                                produce_pair()

                        # ---- B it5 (V heads 0-3) + pre-produce ----------
                        with tc.tile_pool(name="psB5", bufs=4,
                                          space="PSUM") as psB5:
                            for ec in range(6):
                                emit_b_tile(5, ec, pB, psB5)
                                fill_pts()
                            for hv in range(4):
                                load_v(hv)


                        # ---- consume stream; B it6/it7 spliced into the
                        # first 12 steps (B tiles gate nothing the exps
                        # need, so no PE-FIFO circularity; V loads for h>=4
                        # trigger naturally after their B tiles).
                        with tc.tile_pool(name="ops", bufs=1,
                                          space="PSUM") as opsum:

                            def consume_step(s):
                                h, jt = steps[s]
                                po, hf = 64 * (h % 2), h // 2
                                if jt == 0:
                                    ops0 = opsum.tile([128, 512], F32,
                                                      tag="ops0")
                                    ops1 = opsum.tile([128, 512], F32,
                                                      tag="ops1")
                                    state["ops"] = (ops0, ops1)
                                ops = state["ops"]
                                assert pts, f"pt queue empty at step {s}"
                                pt_cur = pts.pop(0)
                                fill_pts()
                                for ic in range(2):
                                    nc.tensor.matmul(
                                        ops[ic],
                                        lhsT=vhs[h % 4][:, jt, :],
                                        rhs=pt_cur[:, ic, :],
                                        start=(jt == 0), stop=(jt == 7),
                                        skip_group_check=True)
                                if jt == 7 and h + 4 < H:
                                    load_v(h + 4)
                                if jt == 7:
                                    # spill [O; Z] to SBUF to free the ops
                                    # tags fast; recip+mul off the critical
                                    # path (custom recip needs UNSHIFTED
                                    # partitions — regular copies shift).
                                    spz = sppool.tile([64, 2, 512], F32,
                                                      tag="spz")
                                    spo = sppool.tile([64, 2, 512], BF,
                                                      tag="spo")
                                    for ic in range(2):
                                        nc.vector.tensor_copy(
                                            spz[:, ic, :], ops[ic][64:128, :])
                                        nc.vector.tensor_copy(
                                            spo[:, ic, :], ops[ic][0:64, :])
                                    rzs = rzpool.tile([64, 2, 512], F32,
                                                      tag="rzs")
                                    nc.vector.reciprocal_approx_fast(rzs, spz)
                                    nc.vector.tensor_mul(
                                        oT[po:po + 64, hf, :].rearrange(
                                            "p (i c) -> p i c", c=512),
                                        spo, rzs)

                            s = 0
                            # clean consume ramp for steps 0-11, then B
                            # it6/it7 spliced 1-per-2-steps (the produce
                            # queue absorbs the PE debt without ACT gaps).
                            for _ in range(12):
                                consume_step(s); s += 1
                            with tc.tile_pool(name="psB2", bufs=2,
                                              space="PSUM") as psB2:
                                for u in range(12):
                                    consume_step(s); s += 1
                                    emit_b_tile(6 + u // 6, u % 6, pB, psB2)
                                    consume_step(s); s += 1
                            while s < len(steps):
                                consume_step(s); s += 1

                # ---------- phase 3: out = oT.T @ w_out^T + b ----------
                with tc.tile_pool(name="p3st", bufs=4) as p3st, \
                     tc.tile_pool(name="ps3", bufs=4, space="PSUM") as ps3p:
                    # HAM warm-keepers: cover the PE idle while the last
                    # head's divide chain runs on DVE, so the out-projection
                    # starts at full clock.
                    wps = ps3p.tile([128, 512], F32, tag="warm")
                    for _ in range(3):
                        nc.tensor.matmul(wps, lhsT=WOT[:, 0, 0:128],
                                         rhs=WOT[:, 0, 0:512],
                                         start=True, stop=True)
                    for it in range(8):
                        for ec in range(2):
                            rps = ps3p.tile([128, 512], F32)
                            for ct in range(8):
                                nc.tensor.matmul(
                                    rps,
                                    lhsT=oT[:, ct, it * 128:(it + 1) * 128],
                                    rhs=WOT[:, ct, ec * 512:(ec + 1) * 512],
                                    start=(ct == 0), stop=(ct == 7))
                            ost = p3st.tile([128, 512], F32)
                            nc.vector.tensor_add(
                                ost, rps, biasrep[:, ec * 512:(ec + 1) * 512])
                            nc.sync.dma_start(
                                out=outp[it * 128:(it + 1) * 128,
                                         ec * 512:(ec + 1) * 512],
                                in_=ost)

    nc.finalize()
    return nc


_CACHE = {}


def _get_nc():
    if "nc" not in _CACHE:
        _CACHE["nc"] = build()
    return _CACHE["nc"]


def make_in_maps(x, w_qkv, w_out, b_out):
    bf = ml_dtypes.bfloat16
    wqkvt = np.ascontiguousarray(np.asarray(w_qkv, dtype=np.float32).T).astype(bf)
    woutt = np.ascontiguousarray(np.asarray(w_out, dtype=np.float32).T).astype(bf)
    bias = np.ascontiguousarray(
        np.asarray(b_out, dtype=np.float32).reshape(1, DIM)).astype(bf)
    x = np.asarray(x, dtype=np.float32)
    return [
        {
            "xt": np.ascontiguousarray(x[b].T).astype(bf),
            "wqkvt": wqkvt,
            "woutt": woutt,
            "bias": bias,
        }
        for b in range(N_CORES)
    ]


def kernel(x, w_qkv, w_out, b_out):
    nc = _get_nc()
    in_maps = make_in_maps(x, w_qkv, w_out, b_out)
    res = run_bass_kernel_spmd(nc, in_maps, core_ids=list(range(N_CORES)))
    return np.stack(
        [res.results[b]["out"] for b in range(N_CORES)], axis=0
    ).astype(np.float32)
